# revision 69
# baseline (speedup 1.0000x reference)
"""Windowed multi-head attention (DWAttention) Bass kernel for Trainium2.

Problem: x[B=2, n=64, N=256, C=384] -> per-window MHA (H=12, d=32) with fused
QKV projection + out_proj (no bias on out_proj, in_proj bias provided).

Strategy (8 NeuronCores, data-parallel over the B*n = 128 independent
windows -> 16 windows per core).

Cost-model facts driving the design (TRN2):
  * matmul time = out-free-size x cycles/row, independent of K and M.
  * bf16 runs 1 cycle/row; fp8e4/e5 with perf_mode=DoubleRow runs 0.5
    cycles/row (two K-subtiles, indexed along a free dim of each operand,
    accumulated in one pass).
  * GpSimd (Pool) engine cannot touch PSUM; DVE/Act PSUM access costs a
    fixed ~125/185ns per instruction.
  * Act exp runs 1 elem/cycle @1.2GHz: 6144 lane-cycles per window makes
    the Activation engine the overall bottleneck (~6.2us/window, ~100us
    floor for 16 windows) once the scores run in fp8.
  * One shared HWDGE generator round-robins the per-engine DMA queues
    (~625ns/descriptor-gen); Pool issues DMAs via software DGE instead.
  * Transfer-complete semaphores cost a fixed ~900ns.

Per window w (tokens N=256, channels C=384, 3 chan-tiles of 128):
  1. qk^T = W_qk @ x^T: 6 chan-major psum tiles [128,512] (window pair),
     bf16 mms; DVE bias-add evicts to FP8E4 qk tile [128, 7, 512] whose
     7th chunk is memset zero.  Scores q,k in e4m3 cost ~1.5e-2 rel err
     (tolerance 2e-2), measured on HW.
  2. Scores per head h: ONE DoubleRow matmul per (h, k-tile): subtile
     pair = (k_h, zeros) x (q_h, zeros) via stepped-slice APs that land
     the second subtile on the zero chunk: out = k_h^T q_h at 128 cycles
     instead of bf16's 256.  PE drops to ~6.0us/window.
  3. exp on Act per 2-head batch ([128,1024] psum, scale fused) -> A^T
     bf16.  attn@v + ones-denominator accumulate in o psum [128,512].
  4. normalize: recip(den) DVE, raw evict, per-head scale on Pool, DMA
     XBAR transpose -> oT; out-proj from oT, DVE evict, DMA out.

The 16 windows are software-pipelined: slot s emits proj(w=s),
attention(w=s-1), output(w=s-3) interleaved; PSUM = scores 2x2 +
attn-out 2x1 + proj/fin 2x1 = 8 banks exactly.

Because Act is the bottleneck, the pipeline head and tail are flattened
around the exp stream:
  head: a small duplicate of W_qk's first chunk-pair columns (wqkh) plus
    x chunks spread over SP/Act/Pool DMA queues land the first projection
    by ~4us; PE warmup matmuls start the p-state ramp; window 0's score
    batches interleave per proj chunk (scheduler forced via
    tile_wait_until) so the first exp fires at ~6.9us.
  tail: the last window's av batches 4/5 write a separate psum tile
    (o_hi, in a freed score bank) so heads 0-7 normalize/transpose/
    out-proj run during the final exps (dep tracking is tile-granular);
    after the last exp only heads 8-11 remain: one recip + one
    broadcast-AP tensor_tensor (stride-0 in1) + 2 transposes + one Act
    copy + g=2 matmuls + DVE/Act-split eviction + split DMA.  fin(w-2)
    fills the PE between av batches and ships via Pool's software DGE.
"""

import numpy as np
from contextlib import ExitStack

import ml_dtypes

import concourse.bass as bass
import concourse.masks as masks
import concourse.mybir as mybir
import concourse.tile as tile
from concourse import bacc
from concourse.bass_utils import run_bass_kernel_spmd

# Problem constants (hardcoded per contract).
B, NWIN, N, C = 2, 64, 256, 384
H, D = 12, 32
SCALE = float(D) ** -0.5
NCORES = 8
WPC = (B * NWIN) // NCORES  # windows per core = 16
CT = C // 128               # channel tiles = 3
TT = N // 128               # token (q/k) tiles = 2
NB = H // 2                 # exp batches of 2 heads = 6

F32 = mybir.dt.float32
BF16 = mybir.dt.bfloat16
FP8 = mybir.dt.float8e4
DR = mybir.MatmulPerfMode.DoubleRow
ADD = mybir.AluOpType.add
MULT = mybir.AluOpType.mult
EXP = mybir.ActivationFunctionType.Exp


def build_program(wpc=WPC, reps=0):
    """reps>0 wraps the whole per-core body in a hardware loop executing it
    reps times - used only for wall-clock HW timing (outputs unchanged)."""
    nc = bacc.Bacc()

    xt_h = nc.dram_tensor("xt", [wpc // 2, 128, CT, 2 * N], BF16, kind="ExternalInput")
    wqk_h = nc.dram_tensor("wqkt", [128, CT, 2 * C], BF16, kind="ExternalInput")
    # duplicate copy of just the first chunk-pair's W_qk columns (j=0 and
    # j=CT): 1/3 the bytes, loaded first, so the pipeline-head projection
    # starts ~1.5us earlier than waiting for the full wqk chunks
    wqkh_h = nc.dram_tensor("wqkh", [128, CT, 2, 128], BF16, kind="ExternalInput")
    wv_h = nc.dram_tensor("wvt", [128, CT, C], BF16, kind="ExternalInput")
    wo_h = nc.dram_tensor("wot", [128, CT, C], BF16, kind="ExternalInput")
    bqk_h = nc.dram_tensor("bqkt", [128, 2 * CT], F32, kind="ExternalInput")
    bvb_h = nc.dram_tensor("bvb", [128, C], F32, kind="ExternalInput")
    out_h = nc.dram_tensor("out", [wpc, TT, 128, C], F32, kind="ExternalOutput")

    with ExitStack() as ctx:
        tc = ctx.enter_context(tile.TileContext(nc))
        wpool = ctx.enter_context(tc.tile_pool(name="wpool", bufs=1))
        xpool = ctx.enter_context(tc.tile_pool(name="xpool", bufs=4))
        qkpool = ctx.enter_context(tc.tile_pool(name="qkpool", bufs=3))
        vpool = ctx.enter_context(tc.tile_pool(name="vpool", bufs=3))
        apool = ctx.enter_context(tc.tile_pool(name="apool", bufs=6))
        opool = ctx.enter_context(tc.tile_pool(name="opool", bufs=6))
        o2pool = ctx.enter_context(tc.tile_pool(name="o2pool", bufs=6))
        otpool = ctx.enter_context(tc.tile_pool(name="otpool", bufs=4))
        fpool = ctx.enter_context(tc.tile_pool(name="fpool", bufs=4))
        rpool = ctx.enter_context(tc.tile_pool(name="rpool", bufs=6))
        # PSUM: scores 2bufs x 2banks + attn-out 2bufs x 1bank
        #       + proj/fin 2bufs x 1bank = 8 banks exactly.
        sc_ps = ctx.enter_context(tc.tile_pool(name="sc_ps", bufs=2, space="PSUM"))
        o_ps = ctx.enter_context(tc.tile_pool(name="o_ps", bufs=2, space="PSUM"))
        pj_ps = ctx.enter_context(tc.tile_pool(name="pj_ps", bufs=2, space="PSUM"))

        st = {}  # per-window pipeline state

        def dma_x(p):
            """Load x^T for window pair (2p, 2p+1) in one tile: the QK
            projection batches both windows into free=512 matmuls.  Issued
            via the Pool engine's software DGE so these large unblocked
            loads never hog the shared HWDGE generator ahead of
            latency-critical weight/bias loads."""
            xt = xpool.tile([128, CT, 2 * N], BF16, name="xt_sb")
            nc.gpsimd.dma_start(out=xt, in_=xt_h.ap()[p])
            st[2 * p] = {"xt": xt, "par": 0}
            st[2 * p + 1] = {"xt": xt, "par": 1}

        # ---- one-time constants (ordered so the first proj's inputs land
        # first: x0 + wqk per c-tile + qk bias, then the rest; weight DMAs
        # ride the Activation engine's DGE queue, in parallel with SP's;
        # x0 chunks alternate SP/Pool queues so the load halves in time) ----
        xt0 = xpool.tile([128, CT, 2 * N], BF16, name="xt_sb")
        wqk_sb = wpool.tile([128, CT, 2 * C], BF16, name="wqk_sb")
        bqk_sb = wpool.tile([128, 2 * CT], F32, name="bqk_sb")
        # x chunk 1 + the tiny qk bias ride Pool's software DGE; x chunks
        # 0/2 ride SP's HWDGE queue; all weights ride Act's HWDGE queue.
        # The shared HWDGE generator round-robins SP/Act, so this gets
        # every first-projection input landed by ~3.5us.
        nc.gpsimd.dma_start(out=xt0[:, 1, :], in_=xt_h.ap()[0, :, 1, :])
        nc.gpsimd.dma_start(out=bqk_sb, in_=bqk_h.ap())
        wqkh_sb = wpool.tile([128, CT, 2, 128], BF16, name="wqkh_sb")
        # wqkh first on SP so its transfer leads the shared DMA pipe; x
        # chunk 0 takes the Act queue slot (only ONE dma issue on the Act
        # sequencer - more would block the head's Act-side eviction behind
        # ~667ns of SEQ time each)
        nc.sync.dma_start(out=wqkh_sb, in_=wqkh_h.ap())
        nc.scalar.dma_start(out=xt0[:, 0, :], in_=xt_h.ap()[0, :, 0, :])
        nc.sync.dma_start(out=xt0[:, 2, :], in_=xt_h.ap()[0, :, 2, :])
        for c in range(CT):
            nc.sync.dma_start(out=wqk_sb[:, c, :], in_=wqk_h.ap()[:, c, :])
        st[0] = {"xt": xt0, "par": 0}
        st[1] = {"xt": xt0, "par": 1}
        wv_sb = wpool.tile([128, CT, C], BF16, name="wv_sb")
        nc.sync.dma_start(out=wv_sb, in_=wv_h.ap())
        bvb_sb = wpool.tile([128, C], F32, name="bvb_sb")
        nc.sync.dma_start(out=bvb_sb, in_=bvb_h.ap())
        wo_sb = wpool.tile([128, CT, C], BF16, name="wo_sb")
        nc.sync.dma_start(out=wo_sb, in_=wo_h.ap())
        ones_sb = wpool.tile([128, 1], BF16, name="ones_sb")
        nc.vector.memset(ones_sb, 1.0)
        # PE warmup: dependency-free matmuls that keep the tensor engine
        # busy while the first x/w DMAs land, so the p-state ramp (full
        # clock after 3us of continuous execution) is already under way
        # when real work starts.  warm_sb x warm_sb avoids waiting on the
        # Pool-built identity.
        warm_sb = wpool.tile([128, 512], BF16, name="warm_sb")
        nc.vector.memset(warm_sb, 0.0)
        for _ in range(3):
            wps = pj_ps.tile([128, 512], F32, tag="pj", name="warm_ps")
            nc.tensor.matmul(wps, warm_sb[:, 0:128], warm_sb, start=True, stop=True)
        ident_sb = wpool.tile([128, 128], BF16, name="ident_sb")
        masks.make_identity(nc, ident_sb)

        def proj_qk_chunk(w, j0, act_evict=False):
            """Project q/k chan-tiles (j0, j0+CT) for windows (w, w+1) at
            once: the rhs spans the x^T pair tile, free=512 per matmul.

            q/k are evicted to fp8e4 (chunks 0-2 = q, 3-5 = k); chunk 6 is an
            always-zero pad so the score matmuls can run in fp8 DoubleRow mode
            (0.5 cycles/row) with the unused second k-subtile pointed at
            zeros: out = k^T q + 0^T 0."""
            s = st[w]
            if "qk" not in s:
                s["qk"] = qkpool.tile([128, 2 * CT + 1, 2 * N], FP8, name="qk_sb")
                st[w + 1]["qk"] = s["qk"]
                nc.gpsimd.memset(s["qk"][:, 2 * CT, :], 0.0)
            for jj, j in enumerate((j0, j0 + CT)):
                ps = pj_ps.tile([128, 2 * N], F32, tag="pj", name="ps_qk")
                for c in range(CT):
                    if act_evict:  # head: early duplicate weight load
                        lhsT = wqkh_sb[:, c, jj, :]
                    else:
                        lhsT = wqk_sb[:, c, 128 * j:128 * (j + 1)]
                    nc.tensor.matmul(
                        ps, lhsT,
                        s["xt"][:, c, :],
                        start=(c == 0), stop=(c == CT - 1),
                    )
                if act_evict and j >= CT:
                    # pipeline head only: Act is idle before the first exp,
                    # so evicting the k chunk there runs the two evictions
                    # of this chunk pair in parallel instead of serially on
                    # DVE (out = Identity(in*1 + bias), same bias add).
                    with tc.high_priority(offset=54):
                        nc.scalar.activation(
                            out=s["qk"][:, j, :], in_=ps,
                            func=mybir.ActivationFunctionType.Identity,
                            bias=bqk_sb[:, j:j + 1],
                        )
                elif act_evict:
                    with tc.high_priority(offset=54):
                        nc.vector.tensor_scalar(
                            out=s["qk"][:, j, :], in0=ps,
                            scalar1=bqk_sb[:, j:j + 1], scalar2=None, op0=ADD,
                        )
                else:
                    nc.vector.tensor_scalar(
                        out=s["qk"][:, j, :], in0=ps,
                        scalar1=bqk_sb[:, j:j + 1], scalar2=None, op0=ADD,
                    )

        def proj_v(w):
            s = st[w]
            v = vpool.tile([128, TT, C], BF16, name="v_sb")
            s["v"] = v
            for m in range(TT):
                ps = pj_ps.tile([128, C], F32, tag="pj", name="ps_v")
                for c in range(CT):
                    nc.tensor.matmul(
                        ps,
                        s["xt"][:, c, 256 * s["par"] + 128 * m:
                                256 * s["par"] + 128 * (m + 1)],
                        wv_sb[:, c, :],
                        start=(c == 0), stop=(c == CT - 1),
                    )
                with tc.high_priority(offset=-130):
                    # LOWERED priority: the v eviction's consumer (next
                    # window's attn@v) is half a slot away, but at default
                    # priority it sits in the DVE stream right where the
                    # next pair's first qk evictions must run to keep the
                    # exp cadence going across the pair boundary.
                    nc.vector.tensor_tensor(
                        out=v[:, m, :], in0=ps, in1=bvb_sb, op=ADD)

        def sc_batch(w, b):
            """Scores + exp for heads 2b, 2b+1.  The score matmuls run at
            raised scheduler priority: the exp chain paces the whole window,
            so PE should pick them the instant their psum bank frees."""
            s = st[w]
            sc = sc_ps.tile([128, 1024], F32, tag="sc", name="sc_t")
            a = apool.tile([128, 1024], BF16, name="a_sb")
            prio = tc.high_priority(offset=54 if b < 2 else 47)
            prio.__enter__()
            par = 256 * s["par"]
            zc = 2 * CT  # index of the zero pad chunk
            for h2 in range(2):
                h = 2 * b + h2
                jq, base = h // 4, 32 * (h % 4)
                for t in range(TT):
                    # S^T[k-tile t, all q] = k_h[t-tile] @ q_h^T as an fp8
                    # DoubleRow matmul: subtile pair = (k_h, zeros) x
                    # (q_h, zeros), half the cycles of a bf16 matmul.
                    nc.tensor.matmul(
                        sc[:, 512 * h2 + 256 * t: 512 * h2 + 256 * (t + 1)],
                        s["qk"][base:base + 32, CT + jq::zc - (CT + jq),
                                par + 128 * t:par + 128 * (t + 1)],
                        s["qk"][base:base + 32, jq::zc - jq, par:par + N],
                        start=True, stop=True, perf_mode=DR,
                        tile_position=(base, 0),
                    )
            nc.scalar.activation(out=a, in_=sc, func=EXP, scale=SCALE)
            prio.__exit__(None, None, None)
            s.setdefault("a", {})[b] = a

        def av_batch(w, b):
            """attn @ v (token-major) + denominators for heads 2b, 2b+1."""
            s = st[w]
            if "o" not in s:
                s["o"] = [
                    o_ps.tile([128, 512], F32, tag="o", name="o_t") for _ in range(TT)
                ]
            a = s["a"].pop(b)
            # last window: batches 4/5 land in a separate psum tile so the
            # early-normalize reads of heads 0-7 (same psum tile otherwise -
            # dep tracking is tile-granular) don't serialize against them
            split = "o_hi" in s and b >= 4
            for h2 in range(2):
                h = 2 * b + h2
                for qt in range(TT):
                    if split:
                        ocol = s["o_hi"][:, qt, 32 * (h - 8):32 * (h - 7)]
                        dcol = s["o_hi"][:, qt, 128 + (h - 8):129 + (h - 8)]
                    else:
                        ot = s["o"][qt]
                        ocol = ot[:, 32 * h:32 * (h + 1)]
                        dcol = ot[:, C + h:C + h + 1]
                    for t in range(TT):
                        lhsT = a[:, 512 * h2 + 256 * t + 128 * qt:
                                 512 * h2 + 256 * t + 128 * (qt + 1)]
                        nc.tensor.matmul(
                            ocol,
                            lhsT, s["v"][:, t, 32 * h:32 * (h + 1)],
                            start=(t == 0), stop=(t == TT - 1),
                            skip_group_check=True,
                        )
                    for t in range(TT):
                        lhsT = a[:, 512 * h2 + 256 * t + 128 * qt:
                                 512 * h2 + 256 * t + 128 * (qt + 1)]
                        nc.tensor.matmul(
                            dcol,
                            lhsT, ones_sb[:, 0:1],
                            start=(t == 0), stop=(t == TT - 1),
                            skip_group_check=True,
                        )

        def norm_qt(w, qt, onrm, cols, rslice):
            """Normalize o[qt][:, cols] into onrm[:, cols]: reciprocal of
            the dens (DVE), broadcast them x32 on Pool, one DVE
            tensor_tensor straight from psum (no raw-copy eviction)."""
            s = st[w]
            nh = len(range(*rslice.indices(H)))
            r = rpool.tile([128, nh], F32, name="recip_sb")
            nc.vector.reciprocal_approx_fast(
                r, s["o"][qt][:, C + rslice.start:C + rslice.stop])
            nc.vector.tensor_tensor(
                out=onrm[:, cols].rearrange("p (a b) -> p a b", b=32),
                in0=s["o"][qt][:, cols].rearrange("p (a b) -> p a b", b=32),
                in1=r.unsqueeze(2).broadcast_to([128, nh, 32]), op=MULT,
            )

        def norm_transpose(w, fast=False):
            """fast=True (pipeline drain): normalize via the Pool-broadcast
            reciprocal + one DVE tensor_tensor from psum, and transpose on
            the now-idle PE instead of the long-latency DMA XBAR path.
            Steady state keeps the raw-copy + per-head Pool muls: the raw
            copy frees the o psum immediately, which the next window's
            attention accumulation is waiting for."""
            s = st[w]
            oT = otpool.tile([128, CT, N], BF16, name="oT_sb")
            if not fast:
                raws, recips = [], []
                for qt in range(TT):
                    r = rpool.tile([128, H], F32, name="recip_sb")
                    nc.vector.reciprocal_approx_fast(r, s["o"][qt][:, C:C + H])
                    recips.append(r)
                    raw = opool.tile([128, C], BF16, name="oraw_sb")
                    nc.vector.tensor_copy(out=raw, in_=s["o"][qt][:, 0:C])
                    raws.append(raw)
                del s["o"]
                for qt in range(TT):
                    onrm = o2pool.tile([128, C], BF16, name="onrm_sb")
                    for h in range(H):
                        nc.gpsimd.tensor_scalar(
                            out=onrm[:, 32 * h:32 * (h + 1)],
                            in0=raws[qt][:, 32 * h:32 * (h + 1)],
                            scalar1=recips[qt][:, h:h + 1], scalar2=None,
                            op0=MULT,
                        )
                    nc.sync.dma_start_transpose(
                        out=oT[:, :, 128 * qt:128 * (qt + 1)], in_=onrm,
                    )
            else:
                for qt in range(TT):
                    onrm = o2pool.tile([128, C], BF16, name="onrm_sb")
                    norm_qt(w, qt, onrm, slice(0, C), slice(0, H))
                    tr = pj_ps.tile([128, CT, 128], BF16, tag="pj", name="tr_ps")
                    for g in range(CT):
                        nc.tensor.transpose(
                            tr[:, g, :], onrm[:, 128 * g:128 * (g + 1)], ident_sb
                        )
                    nc.vector.tensor_copy(
                        out=oT[:, :, 128 * qt:128 * (qt + 1)], in_=tr
                    )
                del s["o"]
            s["oT"] = oT

        def norm_fast_compute(w):
            """Drain normalize, DVE/Pool stage only: emitted right after the
            window's last av batch so the DVE stream starts it as early as
            possible (its output gates the next pipeline stages)."""
            s = st[w]
            s["onrms"] = []
            for qt in range(TT):
                onrm = o2pool.tile([128, C], BF16, name="onrm_sb")
                s["onrms"].append(onrm)
                norm_qt(w, qt, onrm, slice(0, C), slice(0, H))
            del s["o"]

        def norm_fast_finish(w):
            """Drain normalize, PE stage: transposes + oT copies, emitted at
            slot end so ready score/attention matmuls aren't stuck behind
            them in the in-order PE stream."""
            s = st[w]
            oT = otpool.tile([128, CT, N], BF16, name="oT_sb")
            for qt in range(TT):
                tr = pj_ps.tile([128, CT, 128], BF16, tag="pj", name="tr_ps")
                for g in range(CT):
                    nc.tensor.transpose(
                        tr[:, g, :], s["onrms"][qt][:, 128 * g:128 * (g + 1)],
                        ident_sb,
                    )
                nc.vector.tensor_copy(
                    out=oT[:, :, 128 * qt:128 * (qt + 1)], in_=tr
                )
            s["oT"] = oT

        def last_window_part1(w):
            """Final window, stage 1: attention batch 3, then normalize +
            transpose heads 0-7 (needs only exp batches 0-3) while av
            batches 4/5 accumulate into their own separate psum tile."""
            s = st[w]
            av_batch(w, 3)
            # heads 8-11 + their dens for both q-tiles, in a freed sc bank;
            # av 4/5 write it (not the o tiles part 1 reads), so they run
            # the moment their exps land instead of queueing behind part
            # 1's psum reads
            s["o_hi"] = sc_ps.tile([128, TT, 132], F32, tag="sc", name="ohi_ps")
            av_batch(w, 4)

        def last_window_part1b(w):
            """av batch 5 + normalize/transpose of heads 0-7 (ready after
            exp batch 3).  Runs while the final exps still execute."""
            s = st[w]
            av_batch(w, 5)
            oT = otpool.tile([128, CT, N], BF16, name="oT_sb")
            s["oT"] = oT
            onrm = o2pool.tile([128, TT, C], BF16, name="onrm_sb")
            s["onrm"] = onrm
            trs = pj_ps.tile([128, TT, 2, 128], BF16, tag="pj", name="tr_ps")
            for qt in range(TT):
                norm_qt(w, qt, onrm[:, qt], slice(0, 256), slice(0, 8))
                for g in range(2):
                    nc.tensor.transpose(
                        trs[:, qt, g, :], onrm[:, qt, 128 * g:128 * (g + 1)],
                        ident_sb,
                    )
            with tc.high_priority(offset=-150):
                # deliberately LOWERED priority: this copy only gates the
                # g0/g1 out-proj matmuls (ample slack), and must not sit in
                # the DVE stream ahead of part 2's normalize chain
                nc.vector.tensor_copy(
                    out=oT[:, 0:2, :].rearrange("p g (q c) -> p g q c", q=TT),
                    in_=trs.rearrange("p q g c -> p g q c"),
                )

        def last_window_part2(w):
            """Final window, stage 2: heads 8-11 normalize/transpose, the
            out-proj, eviction and output.  Emitted stage-by-stage (not
            per-qt) so neither qt's chain blocks the other on the in-order
            engines; Act (idle after the last exp) takes the g=2 oT copies
            and half the evictions."""
            s = st[w]
            oT = s["oT"]
            # heads 8-11 normalize straight out of the o_hi psum tile, both
            # q-tiles in one instruction per stage (minimizes the post-exp
            # serial chain: recip -> broadcast-multiply)
            tr2 = sc_ps.tile([128, TT, 128], BF16, tag="sc", name="tr2_ps")
            r2 = rpool.tile([128, TT, 4], F32, name="recip_sb")
            nc.vector.reciprocal_approx_fast(r2, s["o_hi"][:, :, 128:132])
            nc.vector.tensor_tensor(
                out=s["onrm"][:, :, 256:C].rearrange(
                    "p q (h c) -> p q h c", c=32),
                in0=s["o_hi"][:, :, 0:128].rearrange(
                    "p q (h c) -> p q h c", c=32),
                in1=r2.unsqueeze(3).broadcast_to([128, TT, 4, 32]), op=MULT,
            )
            for qt in range(TT):
                nc.tensor.transpose(
                    tr2[:, qt, :], s["onrm"][:, qt, 256:C], ident_sb)
            nc.scalar.activation(
                out=oT[:, 2, :], in_=tr2.rearrange("p q c -> p (q c)"),
                func=mybir.ActivationFunctionType.Copy,
            )
            of = fpool.tile([128, TT, C], F32, name="of_sb")
            for qt in range(TT):
                # out-proj psum from the o pool: its bufs freed when part 1
                # read them, unlike pj which waits on fin(w-1) evictions
                fin = o_ps.tile([128, C], F32, tag="o", name="ps_fin")
                for g in range(CT):
                    nc.tensor.matmul(
                        fin, oT[:, g, 128 * qt:128 * (qt + 1)], wo_sb[:, g, :],
                        start=(g == 0), stop=(g == CT - 1),
                    )
                if qt == 0:
                    nc.vector.tensor_copy(out=of[:, 0, :], in_=fin)
                else:
                    nc.scalar.activation(
                        out=of[:, 1, :], in_=fin,
                        func=mybir.ActivationFunctionType.Copy,
                    )
                nc.sync.dma_start(
                    out=out_h.ap()[w, qt].rearrange("p c -> p c"),
                    in_=of[:, qt, :],
                )
            del s["o"], s["o_hi"]
            st.pop(w)

        def fin_qt(w, qt, act_evict=False):
            s = st[w]
            if "of" not in s:
                s["of"] = fpool.tile([128, TT, C], F32, name="of_sb")
            ps = pj_ps.tile([128, C], F32, tag="pj", name="ps_fin")
            for g in range(CT):
                nc.tensor.matmul(
                    ps,
                    s["oT"][:, g, 128 * qt:128 * (qt + 1)],
                    wo_sb[:, g, :],
                    start=(g == 0), stop=(g == CT - 1),
                )
            if act_evict:  # drain: Act has no exps left
                nc.scalar.activation(
                    out=s["of"][:, qt, :], in_=ps,
                    func=mybir.ActivationFunctionType.Copy,
                )
            else:
                nc.vector.tensor_copy(out=s["of"][:, qt, :], in_=ps)

        def out_dma(w, split=False, pool_q=False):
            s = st.pop(w)
            if split:  # drain: ship each q-tile independently
                eng = nc.gpsimd if pool_q else nc.sync
                for m in range(TT):
                    eng.dma_start(
                        out=out_h.ap()[w, m].rearrange("p c -> p c"),
                        in_=s["of"][:, m, :],
                    )
            else:
                nc.sync.dma_start(
                    out=out_h.ap()[w].rearrange("m p c -> p m c"), in_=s["of"]
                )

        loop_ctx = tc.For_i(0, reps) if reps else None
        if loop_ctx is not None:
            ctx.enter_context(loop_ctx)
            dma_x(0)  # body-local x(0) load for the hardware-loop timing mode

        for si in range(wpc + 2):
            # wp: proj window, wa: attention window (batches 0-3; its batches
            # 4-5 + normalize run at the START of the next slot, after their
            # exps have finished), wo: output window.
            wp, wa, wn, wo = si, si - 1, si - 2, si - 3
            vp = wp < wpc
            va = 0 <= wa < wpc
            vn = 0 <= wn < wpc
            vo = 0 <= wo < wpc
            if vp and wp + 1 < wpc and (wp + 1) % 2 == 0:
                dma_x((wp + 1) // 2)
            fast_n = vn and wn >= wpc - 3
            if si == 0:
                # pipeline head: emit window 0's score batches right after
                # the proj chunk that provides their q/k rows (chunk pair
                # (j, CT+j) serves heads 4j..4j+3 = batches 2j, 2j+1), so
                # the first exp starts ~4us earlier than if all scores
                # waited for the full pair projection.
                proj_qk_chunk(0, 0, act_evict=True)
                sc_batch(0, 0)
                sc_batch(0, 1)
                # hold the rest of the slot back so the scheduler doesn't
                # slot these matmuls ahead of the first score batches in
                # the in-order PE stream (it mispredicts the eviction
                # completion times)
                with tc.tile_wait_until(0.0058):
                    proj_qk_chunk(0, 1)
                    sc_batch(0, 2)
                    sc_batch(0, 3)
                with tc.tile_wait_until(0.0068):
                    proj_qk_chunk(0, 2)
                    sc_batch(0, 4)
                    sc_batch(0, 5)
                    proj_v(0)
                continue
            if si == wpc + 1:
                # pipeline tail: stage 1 of the last window overlaps the
                # final exps; the second-to-last window's output (Act
                # evictions - its exps are done) sits between so its pj
                # bufs recycle to the last window's out-proj in time.
                last_window_part1(wn)
                # fin(14)'s matmuls fill the PE between av batches 4 and 5;
                # its output rides Pool's software DGE so the HWDGE stays
                # clear for the final window's output
                fin_qt(wo, 0, act_evict=True)
                fin_qt(wo, 1, act_evict=True)
                out_dma(wo, split=True, pool_q=True)
                last_window_part1b(wn)
                last_window_part2(wn)
                continue
            if vn:
                av_batch(wn, 3)
                av_batch(wn, 4)
                av_batch(wn, 5)
                if fast_n:
                    norm_fast_compute(wn)
            # window 0's score batches 2-5 were already emitted in slot 0
            va_sc = va and si != 1
            if vp and wp % 2 == 0:
                proj_qk_chunk(wp, 0)
            if va_sc:
                sc_batch(wa, 2)
            if vn and not fast_n:
                # after pj0 so DVE's qk evictions aren't stuck behind the
                # norm chain's recip + raw-o evictions
                norm_transpose(wn)
            if vp and wp % 2 == 0:
                proj_qk_chunk(wp, 1)
            if va:
                av_batch(wa, 0)
            if va_sc:
                sc_batch(wa, 3)
            if vo:
                fin_qt(wo, 0)
            if va_sc:
                sc_batch(wa, 4)
                sc_batch(wa, 5)
            if vp and wp % 2 == 0:
                # scores batches 0-1 of the just-projected pair go BEFORE
                # the j2 chunk's matmuls in the PE stream: their sc psum
                # buf frees (exp(wa,4) WAR) before j2's pj buf does
                # (fin(wo,0) eviction), so this order keeps the exp
                # cadence unbroken across the pair boundary.
                sc_batch(wp, 0)
                sc_batch(wp, 1)
                proj_qk_chunk(wp, 2)
            if va:
                av_batch(wa, 1)
            if vo:
                fin_qt(wo, 1)
                out_dma(wo, split=(wo >= wpc - 2))
            if va:
                av_batch(wa, 2)
            if vp:
                proj_v(wp)
                if wp % 2 == 1:
                    sc_batch(wp, 0)
                    sc_batch(wp, 1)
            if fast_n:
                # drain windows: emit after fin/out work so the PE
                # transposes don't block ready output matmuls behind them
                norm_fast_finish(wn)


    nc.compile()
    return nc


_PROGRAM = None


def _get_program():
    global _PROGRAM
    if _PROGRAM is None:
        _PROGRAM = build_program()
    return _PROGRAM


def make_in_maps(x, in_proj_weight, in_proj_bias, out_proj_weight):
    bf16 = ml_dtypes.bfloat16
    x = np.asarray(x, dtype=np.float32)
    in_proj_weight = np.asarray(in_proj_weight, dtype=np.float32)
    in_proj_bias = np.asarray(in_proj_bias, dtype=np.float32)
    out_proj_weight = np.asarray(out_proj_weight, dtype=np.float32)

    W = B * NWIN
    xt = x.reshape(W, N, C).transpose(0, 2, 1)          # [W, C, N]
    # window pairs: [W/2, 128, CT, 2, N] -> [W/2, 128, CT, 2N]
    xt = xt.reshape(W // 2, 2, CT, 128, N).transpose(0, 3, 2, 1, 4)
    xt = np.ascontiguousarray(xt).astype(bf16)
    xt = xt.reshape(NCORES, WPC // 2, 128, CT, 2 * N)

    def chanmajor(wT):  # [C, O] -> [128, CT, O]
        return np.ascontiguousarray(
            wT.reshape(CT, 128, -1).transpose(1, 0, 2)
        ).astype(bf16)

    wqkt = chanmajor(in_proj_weight[:2 * C].T)
    wqkh = np.ascontiguousarray(
        np.stack([wqkt[:, :, 0:128], wqkt[:, :, C:C + 128]], axis=2))
    wvt = chanmajor(in_proj_weight[2 * C:].T)
    wot = chanmajor(out_proj_weight.T)
    bqkt = np.ascontiguousarray(in_proj_bias[:2 * C].reshape(2 * CT, 128).T)
    bvb = np.ascontiguousarray(np.broadcast_to(in_proj_bias[2 * C:], (128, C)))
    return [
        {"xt": xt[i], "wqkt": wqkt, "wqkh": wqkh, "wvt": wvt, "wot": wot,
         "bqkt": bqkt, "bvb": bvb}
        for i in range(NCORES)
    ]


def assemble_out(results):
    outs = [r["out"].reshape(WPC, N, C) for r in results]
    return np.concatenate(outs).reshape(B, NWIN, N, C).astype(np.float32)


def kernel(x, in_proj_weight, in_proj_bias, out_proj_weight):
    nc = _get_program()
    in_maps = make_in_maps(x, in_proj_weight, in_proj_bias, out_proj_weight)
    res = run_bass_kernel_spmd(nc, in_maps, core_ids=list(range(NCORES)))
    return assemble_out(res.results)



# revision 72
# speedup vs baseline: 1.0002x; 1.0002x over previous
"""Windowed multi-head attention (DWAttention) Bass kernel for Trainium2.

Problem: x[B=2, n=64, N=256, C=384] -> per-window MHA (H=12, d=32) with fused
QKV projection + out_proj (no bias on out_proj, in_proj bias provided).

Strategy (8 NeuronCores, data-parallel over the B*n = 128 independent
windows -> 16 windows per core).

Cost-model facts driving the design (TRN2):
  * matmul time = out-free-size x cycles/row, independent of K and M.
  * bf16 runs 1 cycle/row; fp8e4/e5 with perf_mode=DoubleRow runs 0.5
    cycles/row (two K-subtiles, indexed along a free dim of each operand,
    accumulated in one pass).
  * GpSimd (Pool) engine cannot touch PSUM; DVE/Act PSUM access costs a
    fixed ~125/185ns per instruction.
  * Act exp runs 1 elem/cycle @1.2GHz: 6144 lane-cycles per window makes
    the Activation engine the overall bottleneck (~6.2us/window, ~100us
    floor for 16 windows) once the scores run in fp8.
  * One shared HWDGE generator round-robins the per-engine DMA queues
    (~625ns/descriptor-gen); Pool issues DMAs via software DGE instead.
  * Transfer-complete semaphores cost a fixed ~900ns.

Per window w (tokens N=256, channels C=384, 3 chan-tiles of 128):
  1. qk^T = W_qk @ x^T: 6 chan-major psum tiles [128,512] (window pair),
     bf16 mms; DVE bias-add evicts to FP8E4 qk tile [128, 7, 512] whose
     7th chunk is memset zero.  Scores q,k in e4m3 cost ~1.5e-2 rel err
     (tolerance 2e-2), measured on HW.
  2. Scores per head h: ONE DoubleRow matmul per (h, k-tile): subtile
     pair = (k_h, zeros) x (q_h, zeros) via stepped-slice APs that land
     the second subtile on the zero chunk: out = k_h^T q_h at 128 cycles
     instead of bf16's 256.  PE drops to ~6.0us/window.
  3. exp on Act per 2-head batch ([128,1024] psum, scale fused) -> A^T
     bf16.  attn@v + ones-denominator accumulate in o psum [128,512].
  4. normalize: recip(den) DVE, raw evict, per-head scale on Pool, DMA
     XBAR transpose -> oT; out-proj from oT, DVE evict, DMA out.

The 16 windows are software-pipelined: slot s emits proj(w=s),
attention(w=s-1), output(w=s-3) interleaved; PSUM = scores 2x2 +
attn-out 2x1 + proj/fin 2x1 = 8 banks exactly.

Because Act is the bottleneck, the pipeline head and tail are flattened
around the exp stream:
  head: a small duplicate of W_qk's first chunk-pair columns (wqkh) plus
    x chunks spread over SP/Act/Pool DMA queues land the first projection
    by ~4us; PE warmup matmuls start the p-state ramp; window 0's score
    batches interleave per proj chunk (scheduler forced via
    tile_wait_until) so the first exp fires at ~6.9us.
  tail: the last window's av batches 4/5 write a separate psum tile
    (o_hi, in a freed score bank) so heads 0-7 normalize/transpose/
    out-proj run during the final exps (dep tracking is tile-granular);
    after the last exp only heads 8-11 remain: one recip + one
    broadcast-AP tensor_tensor (stride-0 in1) + 2 transposes + one Act
    copy + g=2 matmuls + DVE/Act-split eviction + split DMA.  fin(w-2)
    fills the PE between av batches and ships via Pool's software DGE.
"""

import numpy as np
from contextlib import ExitStack

import ml_dtypes

import concourse.bass as bass
import concourse.masks as masks
import concourse.mybir as mybir
import concourse.tile as tile
from concourse import bacc
from concourse.bass_utils import run_bass_kernel_spmd

# Problem constants (hardcoded per contract).
B, NWIN, N, C = 2, 64, 256, 384
H, D = 12, 32
SCALE = float(D) ** -0.5
NCORES = 8
WPC = (B * NWIN) // NCORES  # windows per core = 16
CT = C // 128               # channel tiles = 3
TT = N // 128               # token (q/k) tiles = 2
NB = H // 2                 # exp batches of 2 heads = 6

F32 = mybir.dt.float32
BF16 = mybir.dt.bfloat16
FP8 = mybir.dt.float8e4
DR = mybir.MatmulPerfMode.DoubleRow
ADD = mybir.AluOpType.add
MULT = mybir.AluOpType.mult
EXP = mybir.ActivationFunctionType.Exp


def build_program(wpc=WPC, reps=0):
    """reps>0 wraps the whole per-core body in a hardware loop executing it
    reps times - used only for wall-clock HW timing (outputs unchanged)."""
    nc = bacc.Bacc()

    xt_h = nc.dram_tensor("xt", [wpc // 2, 128, CT, 2 * N], BF16, kind="ExternalInput")
    wqk_h = nc.dram_tensor("wqkt", [128, CT, 2 * C], BF16, kind="ExternalInput")
    # duplicate copy of just the first chunk-pair's W_qk columns (j=0 and
    # j=CT): 1/3 the bytes, loaded first, so the pipeline-head projection
    # starts ~1.5us earlier than waiting for the full wqk chunks
    wqkh_h = nc.dram_tensor("wqkh", [128, CT, 2, 128], BF16, kind="ExternalInput")
    wv_h = nc.dram_tensor("wvt", [128, CT, C], BF16, kind="ExternalInput")
    wo_h = nc.dram_tensor("wot", [128, CT, C], BF16, kind="ExternalInput")
    bqk_h = nc.dram_tensor("bqkt", [128, 2 * CT], F32, kind="ExternalInput")
    bvb_h = nc.dram_tensor("bvb", [128, C], F32, kind="ExternalInput")
    out_h = nc.dram_tensor("out", [wpc, TT, 128, C], F32, kind="ExternalOutput")

    with ExitStack() as ctx:
        tc = ctx.enter_context(tile.TileContext(nc))
        wpool = ctx.enter_context(tc.tile_pool(name="wpool", bufs=1))
        xpool = ctx.enter_context(tc.tile_pool(name="xpool", bufs=5))
        qkpool = ctx.enter_context(tc.tile_pool(name="qkpool", bufs=4))
        vpool = ctx.enter_context(tc.tile_pool(name="vpool", bufs=4))
        apool = ctx.enter_context(tc.tile_pool(name="apool", bufs=8))
        opool = ctx.enter_context(tc.tile_pool(name="opool", bufs=8))
        o2pool = ctx.enter_context(tc.tile_pool(name="o2pool", bufs=8))
        otpool = ctx.enter_context(tc.tile_pool(name="otpool", bufs=6))
        fpool = ctx.enter_context(tc.tile_pool(name="fpool", bufs=6))
        rpool = ctx.enter_context(tc.tile_pool(name="rpool", bufs=8))
        # PSUM: scores 2bufs x 2banks + attn-out 2bufs x 1bank
        #       + proj/fin 2bufs x 1bank = 8 banks exactly.
        sc_ps = ctx.enter_context(tc.tile_pool(name="sc_ps", bufs=2, space="PSUM"))
        o_ps = ctx.enter_context(tc.tile_pool(name="o_ps", bufs=2, space="PSUM"))
        pj_ps = ctx.enter_context(tc.tile_pool(name="pj_ps", bufs=2, space="PSUM"))

        st = {}  # per-window pipeline state

        def dma_x(p):
            """Load x^T for window pair (2p, 2p+1) in one tile: the QK
            projection batches both windows into free=512 matmuls.  Issued
            via the Pool engine's software DGE so these large unblocked
            loads never hog the shared HWDGE generator ahead of
            latency-critical weight/bias loads."""
            xt = xpool.tile([128, CT, 2 * N], BF16, name="xt_sb")
            nc.gpsimd.dma_start(out=xt, in_=xt_h.ap()[p])
            st[2 * p] = {"xt": xt, "par": 0}
            st[2 * p + 1] = {"xt": xt, "par": 1}

        # ---- one-time constants (ordered so the first proj's inputs land
        # first: x0 + wqk per c-tile + qk bias, then the rest; weight DMAs
        # ride the Activation engine's DGE queue, in parallel with SP's;
        # x0 chunks alternate SP/Pool queues so the load halves in time) ----
        xt0 = xpool.tile([128, CT, 2 * N], BF16, name="xt_sb")
        wqk_sb = wpool.tile([128, CT, 2 * C], BF16, name="wqk_sb")
        bqk_sb = wpool.tile([128, 2 * CT], F32, name="bqk_sb")
        # x chunk 1 + the tiny qk bias ride Pool's software DGE; x chunks
        # 0/2 ride SP's HWDGE queue; all weights ride Act's HWDGE queue.
        # The shared HWDGE generator round-robins SP/Act, so this gets
        # every first-projection input landed by ~3.5us.
        nc.gpsimd.dma_start(out=xt0[:, 1, :], in_=xt_h.ap()[0, :, 1, :])
        nc.gpsimd.dma_start(out=bqk_sb, in_=bqk_h.ap())
        wqkh_sb = wpool.tile([128, CT, 2, 128], BF16, name="wqkh_sb")
        # wqkh first on SP so its transfer leads the shared DMA pipe; x
        # chunk 0 takes the Act queue slot (only ONE dma issue on the Act
        # sequencer - more would block the head's Act-side eviction behind
        # ~667ns of SEQ time each)
        nc.sync.dma_start(out=wqkh_sb, in_=wqkh_h.ap())
        nc.scalar.dma_start(out=xt0[:, 0, :], in_=xt_h.ap()[0, :, 0, :])
        nc.sync.dma_start(out=xt0[:, 2, :], in_=xt_h.ap()[0, :, 2, :])
        for c in range(CT):
            nc.sync.dma_start(out=wqk_sb[:, c, :], in_=wqk_h.ap()[:, c, :])
        st[0] = {"xt": xt0, "par": 0}
        st[1] = {"xt": xt0, "par": 1}
        wv_sb = wpool.tile([128, CT, C], BF16, name="wv_sb")
        nc.sync.dma_start(out=wv_sb, in_=wv_h.ap())
        bvb_sb = wpool.tile([128, C], F32, name="bvb_sb")
        nc.sync.dma_start(out=bvb_sb, in_=bvb_h.ap())
        wo_sb = wpool.tile([128, CT, C], BF16, name="wo_sb")
        nc.sync.dma_start(out=wo_sb, in_=wo_h.ap())
        ones_sb = wpool.tile([128, 1], BF16, name="ones_sb")
        nc.vector.memset(ones_sb, 1.0)
        # PE warmup: dependency-free matmuls that keep the tensor engine
        # busy while the first x/w DMAs land, so the p-state ramp (full
        # clock after 3us of continuous execution) is already under way
        # when real work starts.  warm_sb x warm_sb avoids waiting on the
        # Pool-built identity.
        warm_sb = wpool.tile([128, 512], BF16, name="warm_sb")
        nc.vector.memset(warm_sb, 0.0)
        for _ in range(3):
            wps = pj_ps.tile([128, 512], F32, tag="pj", name="warm_ps")
            nc.tensor.matmul(wps, warm_sb[:, 0:128], warm_sb, start=True, stop=True)
        ident_sb = wpool.tile([128, 128], BF16, name="ident_sb")
        masks.make_identity(nc, ident_sb)

        def proj_qk_chunk(w, j0, act_evict=False):
            """Project q/k chan-tiles (j0, j0+CT) for windows (w, w+1) at
            once: the rhs spans the x^T pair tile, free=512 per matmul.

            q/k are evicted to fp8e4 (chunks 0-2 = q, 3-5 = k); chunk 6 is an
            always-zero pad so the score matmuls can run in fp8 DoubleRow mode
            (0.5 cycles/row) with the unused second k-subtile pointed at
            zeros: out = k^T q + 0^T 0."""
            s = st[w]
            if "qk" not in s:
                s["qk"] = qkpool.tile([128, 2 * CT + 1, 2 * N], FP8, name="qk_sb")
                st[w + 1]["qk"] = s["qk"]
                nc.gpsimd.memset(s["qk"][:, 2 * CT, :], 0.0)
            for jj, j in enumerate((j0, j0 + CT)):
                ps = pj_ps.tile([128, 2 * N], F32, tag="pj", name="ps_qk")
                for c in range(CT):
                    if act_evict:  # head: early duplicate weight load
                        lhsT = wqkh_sb[:, c, jj, :]
                    else:
                        lhsT = wqk_sb[:, c, 128 * j:128 * (j + 1)]
                    nc.tensor.matmul(
                        ps, lhsT,
                        s["xt"][:, c, :],
                        start=(c == 0), stop=(c == CT - 1),
                    )
                if act_evict and j >= CT:
                    # pipeline head only: Act is idle before the first exp,
                    # so evicting the k chunk there runs the two evictions
                    # of this chunk pair in parallel instead of serially on
                    # DVE (out = Identity(in*1 + bias), same bias add).
                    with tc.high_priority(offset=54):
                        nc.scalar.activation(
                            out=s["qk"][:, j, :], in_=ps,
                            func=mybir.ActivationFunctionType.Identity,
                            bias=bqk_sb[:, j:j + 1],
                        )
                elif act_evict:
                    with tc.high_priority(offset=54):
                        nc.vector.tensor_scalar(
                            out=s["qk"][:, j, :], in0=ps,
                            scalar1=bqk_sb[:, j:j + 1], scalar2=None, op0=ADD,
                        )
                else:
                    nc.vector.tensor_scalar(
                        out=s["qk"][:, j, :], in0=ps,
                        scalar1=bqk_sb[:, j:j + 1], scalar2=None, op0=ADD,
                    )

        def proj_v(w):
            s = st[w]
            v = vpool.tile([128, TT, C], BF16, name="v_sb")
            s["v"] = v
            for m in range(TT):
                ps = pj_ps.tile([128, C], F32, tag="pj", name="ps_v")
                for c in range(CT):
                    nc.tensor.matmul(
                        ps,
                        s["xt"][:, c, 256 * s["par"] + 128 * m:
                                256 * s["par"] + 128 * (m + 1)],
                        wv_sb[:, c, :],
                        start=(c == 0), stop=(c == CT - 1),
                    )
                with tc.high_priority(offset=-130):
                    # LOWERED priority: the v eviction's consumer (next
                    # window's attn@v) is half a slot away, but at default
                    # priority it sits in the DVE stream right where the
                    # next pair's first qk evictions must run to keep the
                    # exp cadence going across the pair boundary.
                    nc.vector.tensor_tensor(
                        out=v[:, m, :], in0=ps, in1=bvb_sb, op=ADD)

        def sc_batch(w, b):
            """Scores + exp for heads 2b, 2b+1.  The score matmuls run at
            raised scheduler priority: the exp chain paces the whole window,
            so PE should pick them the instant their psum bank frees."""
            s = st[w]
            sc = sc_ps.tile([128, 1024], F32, tag="sc", name="sc_t")
            a = apool.tile([128, 1024], BF16, name="a_sb")
            prio = tc.high_priority(offset=54 if b < 2 else 47)
            prio.__enter__()
            par = 256 * s["par"]
            zc = 2 * CT  # index of the zero pad chunk
            for h2 in range(2):
                h = 2 * b + h2
                jq, base = h // 4, 32 * (h % 4)
                for t in range(TT):
                    # S^T[k-tile t, all q] = k_h[t-tile] @ q_h^T as an fp8
                    # DoubleRow matmul: subtile pair = (k_h, zeros) x
                    # (q_h, zeros), half the cycles of a bf16 matmul.
                    nc.tensor.matmul(
                        sc[:, 512 * h2 + 256 * t: 512 * h2 + 256 * (t + 1)],
                        s["qk"][base:base + 32, CT + jq::zc - (CT + jq),
                                par + 128 * t:par + 128 * (t + 1)],
                        s["qk"][base:base + 32, jq::zc - jq, par:par + N],
                        start=True, stop=True, perf_mode=DR,
                        tile_position=(base, 0),
                    )
            nc.scalar.activation(out=a, in_=sc, func=EXP, scale=SCALE)
            prio.__exit__(None, None, None)
            s.setdefault("a", {})[b] = a

        def av_batch(w, b):
            """attn @ v (token-major) + denominators for heads 2b, 2b+1."""
            s = st[w]
            if "o" not in s:
                s["o"] = [
                    o_ps.tile([128, 512], F32, tag="o", name="o_t") for _ in range(TT)
                ]
            a = s["a"].pop(b)
            # last window: batches 4/5 land in a separate psum tile so the
            # early-normalize reads of heads 0-7 (same psum tile otherwise -
            # dep tracking is tile-granular) don't serialize against them
            split = "o_hi" in s and b >= 4
            for h2 in range(2):
                h = 2 * b + h2
                for qt in range(TT):
                    if split:
                        ocol = s["o_hi"][:, qt, 32 * (h - 8):32 * (h - 7)]
                        dcol = s["o_hi"][:, qt, 128 + (h - 8):129 + (h - 8)]
                    else:
                        ot = s["o"][qt]
                        ocol = ot[:, 32 * h:32 * (h + 1)]
                        dcol = ot[:, C + h:C + h + 1]
                    for t in range(TT):
                        lhsT = a[:, 512 * h2 + 256 * t + 128 * qt:
                                 512 * h2 + 256 * t + 128 * (qt + 1)]
                        nc.tensor.matmul(
                            ocol,
                            lhsT, s["v"][:, t, 32 * h:32 * (h + 1)],
                            start=(t == 0), stop=(t == TT - 1),
                            skip_group_check=True,
                        )
                    for t in range(TT):
                        lhsT = a[:, 512 * h2 + 256 * t + 128 * qt:
                                 512 * h2 + 256 * t + 128 * (qt + 1)]
                        nc.tensor.matmul(
                            dcol,
                            lhsT, ones_sb[:, 0:1],
                            start=(t == 0), stop=(t == TT - 1),
                            skip_group_check=True,
                        )

        def norm_qt(w, qt, onrm, cols, rslice):
            """Normalize o[qt][:, cols] into onrm[:, cols]: reciprocal of
            the dens (DVE), broadcast them x32 on Pool, one DVE
            tensor_tensor straight from psum (no raw-copy eviction)."""
            s = st[w]
            nh = len(range(*rslice.indices(H)))
            r = rpool.tile([128, nh], F32, name="recip_sb")
            nc.vector.reciprocal_approx_fast(
                r, s["o"][qt][:, C + rslice.start:C + rslice.stop])
            nc.vector.tensor_tensor(
                out=onrm[:, cols].rearrange("p (a b) -> p a b", b=32),
                in0=s["o"][qt][:, cols].rearrange("p (a b) -> p a b", b=32),
                in1=r.unsqueeze(2).broadcast_to([128, nh, 32]), op=MULT,
            )

        def norm_transpose(w, fast=False):
            """fast=True (pipeline drain): normalize via the Pool-broadcast
            reciprocal + one DVE tensor_tensor from psum, and transpose on
            the now-idle PE instead of the long-latency DMA XBAR path.
            Steady state keeps the raw-copy + per-head Pool muls: the raw
            copy frees the o psum immediately, which the next window's
            attention accumulation is waiting for."""
            s = st[w]
            oT = otpool.tile([128, CT, N], BF16, name="oT_sb")
            if not fast:
                raws, recips = [], []
                for qt in range(TT):
                    r = rpool.tile([128, H], F32, name="recip_sb")
                    nc.vector.reciprocal_approx_fast(r, s["o"][qt][:, C:C + H])
                    recips.append(r)
                    raw = opool.tile([128, C], BF16, name="oraw_sb")
                    nc.vector.tensor_copy(out=raw, in_=s["o"][qt][:, 0:C])
                    raws.append(raw)
                del s["o"]
                for qt in range(TT):
                    onrm = o2pool.tile([128, C], BF16, name="onrm_sb")
                    for h in range(H):
                        nc.gpsimd.tensor_scalar(
                            out=onrm[:, 32 * h:32 * (h + 1)],
                            in0=raws[qt][:, 32 * h:32 * (h + 1)],
                            scalar1=recips[qt][:, h:h + 1], scalar2=None,
                            op0=MULT,
                        )
                    nc.sync.dma_start_transpose(
                        out=oT[:, :, 128 * qt:128 * (qt + 1)], in_=onrm,
                    )
            else:
                for qt in range(TT):
                    onrm = o2pool.tile([128, C], BF16, name="onrm_sb")
                    norm_qt(w, qt, onrm, slice(0, C), slice(0, H))
                    tr = pj_ps.tile([128, CT, 128], BF16, tag="pj", name="tr_ps")
                    for g in range(CT):
                        nc.tensor.transpose(
                            tr[:, g, :], onrm[:, 128 * g:128 * (g + 1)], ident_sb
                        )
                    nc.vector.tensor_copy(
                        out=oT[:, :, 128 * qt:128 * (qt + 1)], in_=tr
                    )
                del s["o"]
            s["oT"] = oT

        def norm_fast_compute(w):
            """Drain normalize, DVE/Pool stage only: emitted right after the
            window's last av batch so the DVE stream starts it as early as
            possible (its output gates the next pipeline stages)."""
            s = st[w]
            s["onrms"] = []
            for qt in range(TT):
                onrm = o2pool.tile([128, C], BF16, name="onrm_sb")
                s["onrms"].append(onrm)
                norm_qt(w, qt, onrm, slice(0, C), slice(0, H))
            del s["o"]

        def norm_fast_finish(w):
            """Drain normalize, PE stage: transposes + oT copies, emitted at
            slot end so ready score/attention matmuls aren't stuck behind
            them in the in-order PE stream."""
            s = st[w]
            oT = otpool.tile([128, CT, N], BF16, name="oT_sb")
            for qt in range(TT):
                tr = pj_ps.tile([128, CT, 128], BF16, tag="pj", name="tr_ps")
                for g in range(CT):
                    nc.tensor.transpose(
                        tr[:, g, :], s["onrms"][qt][:, 128 * g:128 * (g + 1)],
                        ident_sb,
                    )
                nc.vector.tensor_copy(
                    out=oT[:, :, 128 * qt:128 * (qt + 1)], in_=tr
                )
            s["oT"] = oT

        def last_window_part1(w):
            """Final window, stage 1: attention batch 3, then normalize +
            transpose heads 0-7 (needs only exp batches 0-3) while av
            batches 4/5 accumulate into their own separate psum tile."""
            s = st[w]
            av_batch(w, 3)
            # heads 8-11 + their dens for both q-tiles, in a freed sc bank;
            # av 4/5 write it (not the o tiles part 1 reads), so they run
            # the moment their exps land instead of queueing behind part
            # 1's psum reads
            s["o_hi"] = sc_ps.tile([128, TT, 132], F32, tag="sc", name="ohi_ps")
            av_batch(w, 4)

        def last_window_part1b(w):
            """av batch 5 + normalize/transpose of heads 0-7 (ready after
            exp batch 3).  Runs while the final exps still execute."""
            s = st[w]
            av_batch(w, 5)
            oT = otpool.tile([128, CT, N], BF16, name="oT_sb")
            s["oT"] = oT
            onrm = o2pool.tile([128, TT, C], BF16, name="onrm_sb")
            s["onrm"] = onrm
            trs = pj_ps.tile([128, TT, 2, 128], BF16, tag="pj", name="tr_ps")
            for qt in range(TT):
                norm_qt(w, qt, onrm[:, qt], slice(0, 256), slice(0, 8))
                for g in range(2):
                    nc.tensor.transpose(
                        trs[:, qt, g, :], onrm[:, qt, 128 * g:128 * (g + 1)],
                        ident_sb,
                    )
            with tc.high_priority(offset=-150):
                # deliberately LOWERED priority: this copy only gates the
                # g0/g1 out-proj matmuls (ample slack), and must not sit in
                # the DVE stream ahead of part 2's normalize chain
                nc.vector.tensor_copy(
                    out=oT[:, 0:2, :].rearrange("p g (q c) -> p g q c", q=TT),
                    in_=trs.rearrange("p q g c -> p g q c"),
                )

        def last_window_part2(w):
            """Final window, stage 2: heads 8-11 normalize/transpose, the
            out-proj, eviction and output.  Emitted stage-by-stage (not
            per-qt) so neither qt's chain blocks the other on the in-order
            engines; Act (idle after the last exp) takes the g=2 oT copies
            and half the evictions."""
            s = st[w]
            oT = s["oT"]
            # heads 8-11 normalize straight out of the o_hi psum tile, both
            # q-tiles in one instruction per stage (minimizes the post-exp
            # serial chain: recip -> broadcast-multiply)
            tr2 = sc_ps.tile([128, TT, 128], BF16, tag="sc", name="tr2_ps")
            r2 = rpool.tile([128, TT, 4], F32, name="recip_sb")
            nc.vector.reciprocal_approx_fast(r2, s["o_hi"][:, :, 128:132])
            nc.vector.tensor_tensor(
                out=s["onrm"][:, :, 256:C].rearrange(
                    "p q (h c) -> p q h c", c=32),
                in0=s["o_hi"][:, :, 0:128].rearrange(
                    "p q (h c) -> p q h c", c=32),
                in1=r2.unsqueeze(3).broadcast_to([128, TT, 4, 32]), op=MULT,
            )
            for qt in range(TT):
                nc.tensor.transpose(
                    tr2[:, qt, :], s["onrm"][:, qt, 256:C], ident_sb)
            nc.scalar.activation(
                out=oT[:, 2, :], in_=tr2.rearrange("p q c -> p (q c)"),
                func=mybir.ActivationFunctionType.Copy,
            )
            of = fpool.tile([128, TT, C], F32, name="of_sb")
            for qt in range(TT):
                # out-proj psum from the o pool: its bufs freed when part 1
                # read them, unlike pj which waits on fin(w-1) evictions
                fin = o_ps.tile([128, C], F32, tag="o", name="ps_fin")
                for g in range(CT):
                    nc.tensor.matmul(
                        fin, oT[:, g, 128 * qt:128 * (qt + 1)], wo_sb[:, g, :],
                        start=(g == 0), stop=(g == CT - 1),
                    )
                if qt == 0:
                    nc.vector.tensor_copy(out=of[:, 0, :], in_=fin)
                else:
                    nc.scalar.activation(
                        out=of[:, 1, :], in_=fin,
                        func=mybir.ActivationFunctionType.Copy,
                    )
                nc.sync.dma_start(
                    out=out_h.ap()[w, qt].rearrange("p c -> p c"),
                    in_=of[:, qt, :],
                )
            del s["o"], s["o_hi"]
            st.pop(w)

        def fin_qt(w, qt, act_evict=False):
            s = st[w]
            if "of" not in s:
                s["of"] = fpool.tile([128, TT, C], F32, name="of_sb")
            ps = pj_ps.tile([128, C], F32, tag="pj", name="ps_fin")
            for g in range(CT):
                nc.tensor.matmul(
                    ps,
                    s["oT"][:, g, 128 * qt:128 * (qt + 1)],
                    wo_sb[:, g, :],
                    start=(g == 0), stop=(g == CT - 1),
                )
            if act_evict:  # drain: Act has no exps left
                nc.scalar.activation(
                    out=s["of"][:, qt, :], in_=ps,
                    func=mybir.ActivationFunctionType.Copy,
                )
            else:
                nc.vector.tensor_copy(out=s["of"][:, qt, :], in_=ps)

        def out_dma(w, split=False, pool_q=False):
            s = st.pop(w)
            if split:  # drain: ship each q-tile independently
                eng = nc.gpsimd if pool_q else nc.sync
                for m in range(TT):
                    eng.dma_start(
                        out=out_h.ap()[w, m].rearrange("p c -> p c"),
                        in_=s["of"][:, m, :],
                    )
            else:
                nc.sync.dma_start(
                    out=out_h.ap()[w].rearrange("m p c -> p m c"), in_=s["of"]
                )

        loop_ctx = tc.For_i(0, reps) if reps else None
        if loop_ctx is not None:
            ctx.enter_context(loop_ctx)
            dma_x(0)  # body-local x(0) load for the hardware-loop timing mode

        for si in range(wpc + 2):
            # wp: proj window, wa: attention window (batches 0-3; its batches
            # 4-5 + normalize run at the START of the next slot, after their
            # exps have finished), wo: output window.
            wp, wa, wn, wo = si, si - 1, si - 2, si - 3
            vp = wp < wpc
            va = 0 <= wa < wpc
            vn = 0 <= wn < wpc
            vo = 0 <= wo < wpc
            if vp and wp + 1 < wpc and (wp + 1) % 2 == 0:
                dma_x((wp + 1) // 2)
            fast_n = vn and wn >= wpc - 3
            if si == 0:
                # pipeline head: emit window 0's score batches right after
                # the proj chunk that provides their q/k rows (chunk pair
                # (j, CT+j) serves heads 4j..4j+3 = batches 2j, 2j+1), so
                # the first exp starts ~4us earlier than if all scores
                # waited for the full pair projection.
                proj_qk_chunk(0, 0, act_evict=True)
                sc_batch(0, 0)
                sc_batch(0, 1)
                # hold the rest of the slot back so the scheduler doesn't
                # slot these matmuls ahead of the first score batches in
                # the in-order PE stream (it mispredicts the eviction
                # completion times)
                with tc.tile_wait_until(0.0058):
                    proj_qk_chunk(0, 1)
                    sc_batch(0, 2)
                    sc_batch(0, 3)
                with tc.tile_wait_until(0.0068):
                    proj_qk_chunk(0, 2)
                    sc_batch(0, 4)
                    sc_batch(0, 5)
                    proj_v(0)
                continue
            if si == wpc + 1:
                # pipeline tail: stage 1 of the last window overlaps the
                # final exps; the second-to-last window's output (Act
                # evictions - its exps are done) sits between so its pj
                # bufs recycle to the last window's out-proj in time.
                last_window_part1(wn)
                # fin(14)'s matmuls fill the PE between av batches 4 and 5;
                # its output rides Pool's software DGE so the HWDGE stays
                # clear for the final window's output
                fin_qt(wo, 0, act_evict=True)
                fin_qt(wo, 1, act_evict=True)
                out_dma(wo, split=True, pool_q=True)
                last_window_part1b(wn)
                last_window_part2(wn)
                continue
            if vn:
                av_batch(wn, 3)
                av_batch(wn, 4)
                av_batch(wn, 5)
                if fast_n:
                    norm_fast_compute(wn)
            # window 0's score batches 2-5 were already emitted in slot 0
            va_sc = va and si != 1
            if vp and wp % 2 == 0:
                proj_qk_chunk(wp, 0)
            if va_sc:
                sc_batch(wa, 2)
            if vn and not fast_n:
                # after pj0 so DVE's qk evictions aren't stuck behind the
                # norm chain's recip + raw-o evictions
                norm_transpose(wn)
            if vp and wp % 2 == 0:
                proj_qk_chunk(wp, 1)
            if va:
                av_batch(wa, 0)
            if va_sc:
                sc_batch(wa, 3)
            if vo:
                fin_qt(wo, 0)
            if va_sc:
                sc_batch(wa, 4)
                sc_batch(wa, 5)
            if vp and wp % 2 == 0:
                # scores batches 0-1 of the just-projected pair go BEFORE
                # the j2 chunk's matmuls in the PE stream: their sc psum
                # buf frees (exp(wa,4) WAR) before j2's pj buf does
                # (fin(wo,0) eviction), so this order keeps the exp
                # cadence unbroken across the pair boundary.
                sc_batch(wp, 0)
                sc_batch(wp, 1)
                proj_qk_chunk(wp, 2)
            if va:
                av_batch(wa, 1)
            if vo:
                fin_qt(wo, 1)
                out_dma(wo, split=(wo >= wpc - 2))
            if va:
                av_batch(wa, 2)
            if vp:
                proj_v(wp)
                if wp % 2 == 1:
                    sc_batch(wp, 0)
                    sc_batch(wp, 1)
            if fast_n:
                # drain windows: emit after fin/out work so the PE
                # transposes don't block ready output matmuls behind them
                norm_fast_finish(wn)


    nc.compile()
    return nc


_PROGRAM = None


def _get_program():
    global _PROGRAM
    if _PROGRAM is None:
        _PROGRAM = build_program()
    return _PROGRAM


def make_in_maps(x, in_proj_weight, in_proj_bias, out_proj_weight):
    bf16 = ml_dtypes.bfloat16
    x = np.asarray(x, dtype=np.float32)
    in_proj_weight = np.asarray(in_proj_weight, dtype=np.float32)
    in_proj_bias = np.asarray(in_proj_bias, dtype=np.float32)
    out_proj_weight = np.asarray(out_proj_weight, dtype=np.float32)

    W = B * NWIN
    xt = x.reshape(W, N, C).transpose(0, 2, 1)          # [W, C, N]
    # window pairs: [W/2, 128, CT, 2, N] -> [W/2, 128, CT, 2N]
    xt = xt.reshape(W // 2, 2, CT, 128, N).transpose(0, 3, 2, 1, 4)
    xt = np.ascontiguousarray(xt).astype(bf16)
    xt = xt.reshape(NCORES, WPC // 2, 128, CT, 2 * N)

    def chanmajor(wT):  # [C, O] -> [128, CT, O]
        return np.ascontiguousarray(
            wT.reshape(CT, 128, -1).transpose(1, 0, 2)
        ).astype(bf16)

    wqkt = chanmajor(in_proj_weight[:2 * C].T)
    wqkh = np.ascontiguousarray(
        np.stack([wqkt[:, :, 0:128], wqkt[:, :, C:C + 128]], axis=2))
    wvt = chanmajor(in_proj_weight[2 * C:].T)
    wot = chanmajor(out_proj_weight.T)
    bqkt = np.ascontiguousarray(in_proj_bias[:2 * C].reshape(2 * CT, 128).T)
    bvb = np.ascontiguousarray(np.broadcast_to(in_proj_bias[2 * C:], (128, C)))
    return [
        {"xt": xt[i], "wqkt": wqkt, "wqkh": wqkh, "wvt": wvt, "wot": wot,
         "bqkt": bqkt, "bvb": bvb}
        for i in range(NCORES)
    ]


def assemble_out(results):
    outs = [r["out"].reshape(WPC, N, C) for r in results]
    return np.concatenate(outs).reshape(B, NWIN, N, C).astype(np.float32)


def kernel(x, in_proj_weight, in_proj_bias, out_proj_weight):
    nc = _get_program()
    in_maps = make_in_maps(x, in_proj_weight, in_proj_bias, out_proj_weight)
    res = run_bass_kernel_spmd(nc, in_maps, core_ids=list(range(NCORES)))
    return assemble_out(res.results)

